# revision 1
# baseline (speedup 1.0000x reference)
"""DissipativeThetaRINN Trainium2 (Bass/Tile) kernel — 8-core data parallel.

Strategy (pure data parallel, per sharding hint):
  - Batch B=2048 is split across 8 NeuronCores (256 rows/core); the tiny
    controller matrices and value-MLP weights are replicated.
  - On-device layout is transposed: features on SBUF partitions, batch on
    the free dimension.
  - Per timestep the implicit layer w = tanh(Cv x + Dvy y + Dvw w) is run
    as a fixed-point iteration. The batch is split into two 128-column
    chunks so chunk A's tanh (ScalarE) overlaps chunk B's matmuls (PE).
    The constant term is re-folded into PSUM by a second accumulating
    matmul each iteration, so ScalarE only does one Tanh per chunk.
  - The fixed point contracts with factor ~0.47/iter; N_ITERS iterations
    reproduce the reference's 30-iteration result to ~1e-4 (the
    reference's own iterate converges to fp32 noise by ~iteration 20).
  - Matmuls run in fp16 (PSUM accumulates fp32); the x recurrence keeps an
    fp32 accumulator on device, and DT is pre-folded into the recurrence
    weights so fp16 rounding only touches the 0.01-scaled increment.
  - The value MLP (independent of the recurrence) is computed in grouped
    timestep pairs and scheduled into the fixed-point loop's engine gaps.
  - log_stds broadcast and the +b2 value bias are applied host-side during
    output assembly.
"""
import numpy as np
import concourse.bass as bass
import concourse.mybir as mybir
import concourse.tile as tile
from concourse import bacc
from concourse.bass_utils import run_bass_kernel_spmd

dt = mybir.dt
AF = mybir.ActivationFunctionType

# problem shape (hardcoded per contract)
BFULL, TFULL = 2048, 128
S, NL, IN, OUT, H = 16, 128, 32, 8, 64
DT = 0.01
N_CORES = 8
N_ITERS = 11   # fixed-point tanh evaluations per timestep
VG = 2         # value-MLP timestep group


def build_kernel(T=TFULL, B=BFULL // N_CORES, n_iters=N_ITERS):
    nc = bacc.Bacc(None, target_bir_lowering=False)
    f32, f16 = dt.float32, dt.float16
    C = B // 2  # batch chunk width

    obsT16 = nc.dram_tensor("obsT16", [T, IN, B], f16, kind="ExternalInput")
    x0T = nc.dram_tensor("x0T", [S, B], f32, kind="ExternalInput")
    Wdvw = nc.dram_tensor("Wdvw", [NL, NL], f16, kind="ExternalInput")
    Wcd = nc.dram_tensor("Wcd", [S + IN, NL], f16, kind="ExternalInput")
    Wu = nc.dram_tensor("Wu", [S + IN, OUT], f16, kind="ExternalInput")
    Wuw = nc.dram_tensor("Wuw", [NL, OUT], f16, kind="ExternalInput")
    Wx = nc.dram_tensor("Wx", [S + IN, S], f16, kind="ExternalInput")
    Wxw = nc.dram_tensor("Wxw", [NL, S], f16, kind="ExternalInput")
    Wv0 = nc.dram_tensor("Wv0", [IN, H], f16, kind="ExternalInput")
    Wv1 = nc.dram_tensor("Wv1", [2 * H, H], f16, kind="ExternalInput")
    Wv2 = nc.dram_tensor("Wv2", [2 * H, 1], f16, kind="ExternalInput")
    b0v = nc.dram_tensor("b0v", [NL, 1], f32, kind="ExternalInput")
    b1v = nc.dram_tensor("b1v", [NL, 1], f32, kind="ExternalInput")

    u_out = nc.dram_tensor("u_out", [T, OUT, B], f32, kind="ExternalOutput")
    v_out = nc.dram_tensor("v_out", [T, B], f32, kind="ExternalOutput")

    NV = VG * B

    with tile.TileContext(nc) as tc:
        with tc.tile_pool(name="wts", bufs=1) as wts, \
             tc.tile_pool(name="xyp", bufs=3) as xyp, \
             tc.tile_pool(name="wp", bufs=2) as wp, \
             tc.tile_pool(name="iop", bufs=3) as iop, \
             tc.tile_pool(name="vp", bufs=2) as vp, \
             tc.tile_pool(name="pw0", bufs=2, space="PSUM") as pwp0, \
             tc.tile_pool(name="pw1", bufs=2, space="PSUM") as pwp1, \
             tc.tile_pool(name="pxp0", bufs=1, space="PSUM") as pxp0, \
             tc.tile_pool(name="pxp1", bufs=1, space="PSUM") as pxp1, \
             tc.tile_pool(name="pup", bufs=1, space="PSUM") as pup, \
             tc.tile_pool(name="phh", bufs=1, space="PSUM") as php:
            pwp = [pwp0, pwp1]

            def wt(name, dram, shape, dtp):
                tl = wts.tile(shape, dtp, name=name)
                nc.sync.dma_start(tl[:], dram[:])
                return tl
            wdvw = wt("wdvw", Wdvw, [NL, NL], f16)
            wcd = wt("wcd", Wcd, [S + IN, NL], f16)
            wu = wt("wu", Wu, [S + IN, OUT], f16)
            wuw = wt("wuw", Wuw, [NL, OUT], f16)
            wx = wt("wx", Wx, [S + IN, S], f16)
            wxw = wt("wxw", Wxw, [NL, S], f16)
            wv0 = wt("wv0", Wv0, [IN, H], f16)
            wv1 = wt("wv1", Wv1, [2 * H, H], f16)
            wv2 = wt("wv2", Wv2, [2 * H, 1], f16)
            b0 = wt("b0", b0v, [NL, 1], f32)
            b1 = wt("b1", b1v, [NL, 1], f32)

            # xy_h [48,B] f16: rows 0:32 = y^T, rows 32:48 = x^T; xt_r = fp32 x accum
            yst_h = iop.tile([IN, B], f16, name="yst_h0", tag="yst_h")
            nc.sync.dma_start(yst_h[:], obsT16[0])
            xt_r = xyp.tile([S, B], f32, name="xt_r0", tag="xt_r")
            nc.sync.dma_start(xt_r[:], x0T[:])
            xy_h = xyp.tile([S + IN, B], f16, name="xy_h0", tag="xy_h")
            nc.vector.tensor_copy(xy_h[0:IN, :], yst_h[:])
            nc.vector.tensor_copy(xy_h[IN:, :], xt_r[:])

            for t in range(T):
                # ---------- value MLP (grouped over VG timesteps) ----------
                if t % VG == 0:
                    with nc.named_scope(f"value_{t}"):
                        obs_v = vp.tile([IN, NV], f16, name=f"obs_v{t}", tag="obs_v")
                        osrc = obsT16[t:t + VG].transpose([1, 0, 2])
                        nc.sync.dma_start(obs_v[:].rearrange("k (g b) -> k g b", g=VG), osrc)
                        nvc = (NV + 511) // 512
                        ph = php.tile([H, NV], dt.float32, name=f"ph1_{t}", tag="ph")
                        for j in range(nvc):
                            js = slice(j * 512, min((j + 1) * 512, NV))
                            nc.tensor.matmul(ph[:, js], wv0[:], obs_v[:, js], start=True, stop=True)
                        h1 = vp.tile([H, NV], f16, name=f"h1_{t}", tag="h1")
                        nc.scalar.activation(h1[:], ph[:], AF.Tanh, bias=b0[0:H, :])
                        ph2 = php.tile([H, NV], dt.float32, name=f"ph2_{t}", tag="ph")
                        for j in range(nvc):
                            js = slice(j * 512, min((j + 1) * 512, NV))
                            nc.tensor.matmul(ph2[:, js], wv1[0:H, :], h1[:, js], start=True, stop=True)
                        h2 = vp.tile([H, NV], f16, name=f"h2_{t}", tag="h1")
                        nc.scalar.activation(h2[:], ph2[:], AF.Tanh, bias=b1[0:H, :])
                        v_sb = vp.tile([1, NV], f32, name=f"v_sb{t}", tag="v_sb")
                        for j in range(nvc):
                            js = slice(j * 512, min((j + 1) * 512, NV))
                            pv = php.tile([1, 512], dt.float32, name=f"pv{t}_{j}", tag="ph")
                            nc.tensor.matmul(pv[:, 0:js.stop - js.start], wv2[0:H, :], h2[:, js],
                                             start=True, stop=True)
                            nc.vector.tensor_copy(v_sb[:, js], pv[:, 0:js.stop - js.start])
                        nc.sync.dma_start(
                            v_out[t:t + VG].rearrange("g b -> (g b)").unsqueeze(0), v_sb[:])

                # ---------- fixed point, 2-chunk ping-pong ----------
                with nc.named_scope(f"fp_{t}"):
                    if t < T - 1:
                        # prefetch next y into the next xy tile
                        yst_h = iop.tile([IN, B], f16, name=f"ysth{t + 1}", tag="yst_h")
                        nc.sync.dma_start(yst_h[:], obsT16[t + 1])
                        xy_hn = xyp.tile([S + IN, B], f16, name=f"xyh{t + 1}", tag="xy_h")
                        nc.vector.tensor_copy(xy_hn[0:IN, :], yst_h[:])
                    w16 = [None, None]
                    for it in range(n_iters):
                        for c in range(2):
                            cs = slice(c * C, (c + 1) * C)
                            p = pwp[c].tile([NL, C], dt.float32, name=f"pw{t}_{it}_{c}", tag=f"pw{c}")
                            if it == 0:
                                nc.tensor.matmul(p[:], wcd[:], xy_h[:, cs], start=True, stop=True)
                            else:
                                nc.tensor.matmul(p[:], wcd[:], xy_h[:, cs], start=True, stop=False)
                                nc.tensor.matmul(p[:], wdvw[:], w16[c][:], start=False, stop=True)
                            wn = wp.tile([NL, C], f16, name=f"w{t}_{it}_{c}", tag=f"w{c}")
                            nc.scalar.activation(wn[:], p[:], AF.Tanh)
                            w16[c] = wn

                # ---------- x_next (critical path), then u ----------
                with nc.named_scope(f"out_{t}"):
                    if t < T - 1:
                        pxp = [pxp0, pxp1]
                        pxc = []
                        for c in range(2):
                            cs = slice(c * C, (c + 1) * C)
                            px = pxp[c].tile([S, C], dt.float32, name=f"px{t}_{c}", tag=f"px{c}")
                            nc.tensor.matmul(px[:], wx[:], xy_h[:, cs], start=True, stop=False)
                            nc.tensor.matmul(px[:], wxw[:], w16[c][:], start=False, stop=True)
                            # critical: fp16 x for the next step's const folds
                            nc.vector.tensor_add(xy_hn[IN:, cs], px[:], xt_r[:, cs])
                            pxc.append(px)
                        # off-critical: fp32 x accumulator
                        xt_rn = xyp.tile([S, B], f32, name=f"xtr{t + 1}", tag="xt_r")
                        for c in range(2):
                            cs = slice(c * C, (c + 1) * C)
                            nc.vector.tensor_add(xt_rn[:, cs], pxc[c][:], xt_r[:, cs])

                    pu = pup.tile([OUT, B], dt.float32, name=f"pu{t}", tag="pu")
                    nc.tensor.matmul(pu[:], wu[:], xy_h[:], start=True, stop=False)
                    for c in range(2):
                        cs = slice(c * C, (c + 1) * C)
                        nc.tensor.matmul(pu[:, cs], wuw[:], w16[c][:], start=False, stop=True)
                    u_sb = iop.tile([OUT, B], f32, name=f"u_sb{t}", tag="u_sb")
                    nc.vector.tensor_copy(u_sb[:], pu[:])
                    nc.sync.dma_start(u_out[t], u_sb[:])

                    if t < T - 1:
                        xt_r, xy_h = xt_rn, xy_hn

    nc.compile()
    return nc


def host_inputs(inputs, core, n_cores=N_CORES):
    BL = inputs["obs"].shape[0] // n_cores
    sl = slice(core * BL, (core + 1) * BL)
    obs = np.ascontiguousarray(np.asarray(inputs["obs"])[sl].transpose(1, 2, 0))
    x0T = np.ascontiguousarray(np.asarray(inputs["x0"])[sl].T)
    g = lambda k: np.asarray(inputs[k])
    return {
        "obsT16": obs.astype(np.float16),
        "x0T": x0T.astype(np.float32),
        "Wdvw": g("Dvw_T").astype(np.float16),
        "Wcd": np.concatenate([g("Dvy_T"), g("Cv_T")], 0).astype(np.float16),
        "Wu": np.concatenate([g("Duy_T"), g("Cu_T")], 0).astype(np.float16),
        "Wuw": g("Duw_T").astype(np.float16),
        "Wx": np.concatenate([DT * g("By_T"), DT * g("A_T")], 0).astype(np.float16),
        "Wxw": (DT * g("Bw_T")).astype(np.float16),
        "Wv0": g("W0").astype(np.float16),
        "Wv1": np.tile(g("W1"), (2, 1)).astype(np.float16),
        "Wv2": np.tile(g("W2"), (2, 1)).astype(np.float16),
        "b0v": np.tile(g("b0").reshape(H, 1), (2, 1)).astype(np.float32),
        "b1v": np.tile(g("b1").reshape(H, 1), (2, 1)).astype(np.float32),
    }


def assemble_output(results, inputs, n_cores=N_CORES):
    obs = np.asarray(inputs["obs"])
    Bfull, T = obs.shape[0], obs.shape[1]
    BL = Bfull // n_cores
    out = np.empty((Bfull, T, 2 * OUT + 1), np.float32)
    log_stds = np.asarray(inputs["log_stds"], np.float32)
    b2 = np.asarray(inputs["b2"], np.float32)
    for c in range(n_cores):
        sl = slice(c * BL, (c + 1) * BL)
        out[sl, :, :OUT] = results[c]["u_out"].transpose(2, 0, 1)
        out[sl, :, OUT:2 * OUT] = log_stds
        out[sl, :, 2 * OUT:] = results[c]["v_out"].T[:, :, None] + b2
    return out


_NC_CACHE = {}


def _get_nc(T):
    if T not in _NC_CACHE:
        _NC_CACHE[T] = build_kernel(T=T)
    return _NC_CACHE[T]


def run_on_hw(inputs, trace=False):
    """Run the SPMD kernel; returns (full_output, exec_time_ns_or_None)."""
    T = np.asarray(inputs["obs"]).shape[1]
    nc = _get_nc(T)
    in_maps = [host_inputs(inputs, c) for c in range(N_CORES)]
    last_err = None
    for attempt in range(3):
        try:
            res = run_bass_kernel_spmd(nc, in_maps, list(range(N_CORES)), trace=trace)
            return assemble_output(res.results, inputs), res.exec_time_ns
        except Exception as e:  # transient device failures: retry
            last_err = e
    raise last_err


def kernel(**inputs) -> np.ndarray:
    out, _ = run_on_hw(inputs, trace=False)
    return out



# revision 6
# speedup vs baseline: 2.4638x; 2.4638x over previous
"""DissipativeThetaRINN Trainium2 (Bass/Tile) kernel — 8-core data parallel.

Strategy (pure data parallel, per sharding hint):
  - Batch B=2048 is split across 8 NeuronCores (256 rows/core); the tiny
    controller matrices and value-MLP weights are replicated.
  - On-device layout is transposed: features on SBUF partitions, batch on
    the free dimension (256 columns per core).
  - The implicit layer w = tanh(Cv x + Dvy y + Dvw w) is solved with only
    N_ITERS=3 tanh evaluations: iteration 0 uses a linear-solve warm start
    w0 = tanh(M c) with M = (I - g Dvw)^-1, g=0.8, folded host-side into
    the const matmul (zero extra device work).  Each remaining iteration is
    one 256-wide matmul pair into PSUM + one 256-wide tanh.  All matmuls
    span the full 256-column batch — on TRN2 the per-instruction overhead
    (LDWEIGHTS ~100ns + drain ~170ns) dominates 128-col streams, so fewer,
    wider instructions beat a 2-chunk ping-pong.
  - x_next (forward Euler) uses the second-to-last w iterate so the final
    tanh is off the timestep-boundary critical path (error ~1e-5, verified
    in fp16-faithful numpy sim: total rel_l2 ≈ 5.5e-3 vs 2e-2 budget).
  - Matmuls run in fp16 (PSUM accumulates fp32); DT is pre-folded into the
    recurrence weights; the x accumulator stays fp32 on device.
  - The value MLP is computed in groups of 4 timesteps, packed 2-per-128
    partitions with block-diagonal weights, scheduled into engine gaps.
  - log_stds broadcast and the +b2 value bias are applied host-side.
"""
import numpy as np
import concourse.bass as bass
import concourse.mybir as mybir
import concourse.tile as tile
from concourse import bacc
from concourse.bass_utils import run_bass_kernel_spmd

dt = mybir.dt
AF = mybir.ActivationFunctionType

# problem shape (hardcoded per contract)
BFULL, TFULL = 2048, 128
S, NL, IN, OUT, H = 16, 128, 32, 8, 64
DT = 0.01
N_CORES = 8
N_ITERS = 3    # tanh evaluations per timestep (incl. warm-start iteration)
G_INIT = 0.8   # warm-start gain: w0 = tanh((I - g Dvw)^-T c)
VG = 4         # value-MLP timestep group (packed 2x2 onto 128 partitions)


def build_kernel(T=TFULL, B=BFULL // N_CORES, n_iters=N_ITERS):
    nc = bacc.Bacc(None, target_bir_lowering=False)
    f32, f16 = dt.float32, dt.float16
    assert n_iters >= 2

    obsT16 = nc.dram_tensor("obsT16", [T, IN, B], f16, kind="ExternalInput")
    x0T = nc.dram_tensor("x0T", [S, B], f32, kind="ExternalInput")
    Wdvw = nc.dram_tensor("Wdvw", [NL, NL], f16, kind="ExternalInput")
    Wcd = nc.dram_tensor("Wcd", [S + IN, NL], f16, kind="ExternalInput")
    Wcd0 = nc.dram_tensor("Wcd0", [S + IN, NL], f16, kind="ExternalInput")
    Wu = nc.dram_tensor("Wu", [S + IN, OUT], f16, kind="ExternalInput")
    Wuw = nc.dram_tensor("Wuw", [NL, OUT], f16, kind="ExternalInput")
    Wx = nc.dram_tensor("Wx", [S + IN, S], f16, kind="ExternalInput")
    Wxw = nc.dram_tensor("Wxw", [NL, S], f16, kind="ExternalInput")
    Wv0b = nc.dram_tensor("Wv0b", [2 * IN, 2 * H], f16, kind="ExternalInput")
    Wv1b = nc.dram_tensor("Wv1b", [2 * H, 2 * H], f16, kind="ExternalInput")
    Wv2b = nc.dram_tensor("Wv2b", [2 * H, 2], f16, kind="ExternalInput")
    b0v = nc.dram_tensor("b0v", [2 * H, 1], f32, kind="ExternalInput")
    b1v = nc.dram_tensor("b1v", [2 * H, 1], f32, kind="ExternalInput")

    u_out = nc.dram_tensor("u_out", [T, OUT, B], f32, kind="ExternalOutput")
    v_out = nc.dram_tensor("v_out", [T, B], f32, kind="ExternalOutput")

    B2 = 2 * B
    n_groups = (T + VG - 1) // VG

    with tile.TileContext(nc) as tc:
        with tc.tile_pool(name="wts", bufs=1) as wts, \
             tc.tile_pool(name="xyp", bufs=3) as xyp, \
             tc.tile_pool(name="xtp", bufs=2) as xtp, \
             tc.tile_pool(name="wp", bufs=3) as wp, \
             tc.tile_pool(name="obp", bufs=2) as obp, \
             tc.tile_pool(name="vp", bufs=2) as vp, \
             tc.tile_pool(name="up", bufs=2) as up, \
             tc.tile_pool(name="pw", bufs=2, space="PSUM") as pwp, \
             tc.tile_pool(name="px", bufs=2, space="PSUM") as pxp, \
             tc.tile_pool(name="pu", bufs=2, space="PSUM") as pup, \
             tc.tile_pool(name="ph", bufs=1, space="PSUM") as php, \
             tc.tile_pool(name="pv", bufs=1, space="PSUM") as pvp:

            def wt(name, dram, shape, dtp):
                tl = wts.tile(shape, dtp, name=name)
                nc.sync.dma_start(tl[:], dram[:])
                return tl
            wdvw = wt("wdvw", Wdvw, [NL, NL], f16)
            wcd = wt("wcd", Wcd, [S + IN, NL], f16)
            wcd0 = wt("wcd0", Wcd0, [S + IN, NL], f16)
            wu = wt("wu", Wu, [S + IN, OUT], f16)
            wuw = wt("wuw", Wuw, [NL, OUT], f16)
            wx = wt("wx", Wx, [S + IN, S], f16)
            wxw = wt("wxw", Wxw, [NL, S], f16)
            wv0b = wt("wv0b", Wv0b, [2 * IN, 2 * H], f16)
            wv1b = wt("wv1b", Wv1b, [2 * H, 2 * H], f16)
            wv2b = wt("wv2b", Wv2b, [2 * H, 2], f16)
            b0 = wt("b0", b0v, [2 * H, 1], f32)
            b1 = wt("b1", b1v, [2 * H, 1], f32)

            def load_obs4(g):
                """obs4 [64, 2B]: rows 0:32 = ts {4g, 4g+1}, rows 32:64 =
                ts {4g+2, 4g+3} (two col blocks of B each)."""
                t0 = g * VG
                ob = obp.tile([2 * IN, B2], f16, name=f"obs4_{g}", tag="obs4")
                nc.sync.dma_start(
                    ob[0:IN, :].rearrange("k (g1 b) -> k g1 b", g1=2),
                    obsT16[t0:t0 + 2].transpose([1, 0, 2]))
                nc.sync.dma_start(
                    ob[IN:, :].rearrange("k (g1 b) -> k g1 b", g1=2),
                    obsT16[t0 + 2:t0 + 4].transpose([1, 0, 2]))
                return ob

            obs4 = load_obs4(0)
            obs4_next = load_obs4(1) if n_groups > 1 else None

            # xy_h [48, B] f16: rows 0:32 = y^T, rows 32:48 = x^T
            xt_r = xtp.tile([S, B], f32, name="xt_r0", tag="xt_r")
            nc.sync.dma_start(xt_r[:], x0T[:])
            xy_h = xyp.tile([S + IN, B], f16, name="xy_h0", tag="xy_h")
            nc.sync.dma_start(xy_h[0:IN, :], obsT16[0])
            nc.vector.tensor_copy(xy_h[IN:, :], xt_r[:])

            u4 = None
            for t in range(T):
                g, g4 = t // VG, t % VG
                # ---------- value MLP (4-ts group, 2x2 packed) ----------
                if g4 == 0:
                    with nc.named_scope(f"value_{t}"):
                        ph = php.tile([2 * H, B2], f32, name=f"ph_{g}", tag="ph")
                        nc.tensor.matmul(ph[:], wv0b[:], obs4[:], start=True, stop=True)
                        h1 = vp.tile([2 * H, B2], f16, name=f"h1_{g}", tag="h")
                        nc.scalar.activation(h1[:], ph[:], AF.Tanh, bias=b0[:])
                        ph2 = php.tile([2 * H, B2], f32, name=f"ph2_{g}", tag="ph")
                        nc.tensor.matmul(ph2[:], wv1b[:], h1[:], start=True, stop=True)
                        h2 = vp.tile([2 * H, B2], f16, name=f"h2_{g}", tag="h")
                        nc.scalar.activation(h2[:], ph2[:], AF.Tanh, bias=b1[:])
                        pv = pvp.tile([2, B2], f32, name=f"pv_{g}", tag="pv")
                        nc.tensor.matmul(pv[:], wv2b[:], h2[:], start=True, stop=True)
                        v_sb = vp.tile([2, B2], f32, name=f"v_sb{g}", tag="v_sb")
                        nc.vector.tensor_copy(v_sb[:], pv[:])
                        nc.sync.dma_start(
                            v_out[t:t + VG].rearrange("(r g1) b -> r (g1 b)", r=2),
                            v_sb[:])
                    # rotate prefetched group
                    obs4 = obs4_next
                    if g + 2 < n_groups:
                        obs4_next = load_obs4(g + 2)

                # ---------- fixed point: warm start + (n-1) refolds ----------
                with nc.named_scope(f"fp_{t}"):
                    p = pwp.tile([NL, B], f32, name=f"pw{t}_0", tag="pw")
                    nc.tensor.matmul(p[:], wcd0[:], xy_h[:], start=True, stop=True)
                    ws = []
                    for it in range(n_iters):
                        if it > 0:
                            p = pwp.tile([NL, B], f32, name=f"pw{t}_{it}", tag="pw")
                            nc.tensor.matmul(p[:], wcd[:], xy_h[:], start=True, stop=False)
                            nc.tensor.matmul(p[:], wdvw[:], ws[-1][:], start=False, stop=True)
                        wn = wp.tile([NL, B], f16, name=f"w{t}_{it}", tag="w")
                        nc.scalar.activation(wn[:], p[:], AF.Tanh)
                        ws.append(wn)
                    w_x, w_u = ws[-2], ws[-1]

                # ---------- x_next (critical path, uses w_{n-2}), then u ----------
                with nc.named_scope(f"out_{t}"):
                    if t < T - 1:
                        xy_hn = xyp.tile([S + IN, B], f16, name=f"xyh{t + 1}", tag="xy_h")
                        nc.sync.dma_start(xy_hn[0:IN, :], obsT16[t + 1])
                        px = pxp.tile([S, B], f32, name=f"px{t}", tag="px")
                        nc.tensor.matmul(px[:], wx[:], xy_h[:], start=True, stop=False)
                        nc.tensor.matmul(px[:], wxw[:], w_x[:], start=False, stop=True)
                        nc.vector.tensor_add(xy_hn[IN:, :], px[:], xt_r[:])
                        if t < T - 2:
                            xt_rn = xtp.tile([S, B], f32, name=f"xtr{t + 1}", tag="xt_r")
                            nc.vector.tensor_add(xt_rn[:], px[:], xt_r[:])
                        else:
                            xt_rn = xt_r

                    pu = pup.tile([OUT, B], f32, name=f"pu{t}", tag="pu")
                    nc.tensor.matmul(pu[:], wu[:], xy_h[:], start=True, stop=False)
                    nc.tensor.matmul(pu[:], wuw[:], w_u[:], start=False, stop=True)
                    if g4 == 0:
                        u4 = up.tile([OUT, VG * B], f32, name=f"u4_{g}", tag="u4")
                    nc.vector.tensor_copy(u4[:, g4 * B:(g4 + 1) * B], pu[:])
                    if g4 == VG - 1:
                        nc.sync.dma_start(
                            u_out[t - VG + 1:t + 1].transpose([1, 0, 2]),
                            u4[:].rearrange("o (g1 b) -> o g1 b", g1=VG))

                    if t < T - 1:
                        xt_r, xy_h = xt_rn, xy_hn

    nc.compile()
    return nc


def host_inputs(inputs, core, n_cores=N_CORES):
    BL = inputs["obs"].shape[0] // n_cores
    sl = slice(core * BL, (core + 1) * BL)
    obs = np.ascontiguousarray(np.asarray(inputs["obs"])[sl].transpose(1, 2, 0))
    x0T = np.ascontiguousarray(np.asarray(inputs["x0"])[sl].T)
    g = lambda k: np.asarray(inputs[k]).astype(np.float32)
    Dvw = g("Dvw_T")
    M = np.linalg.inv(np.eye(NL, dtype=np.float32) - G_INIT * Dvw)
    Wcd = np.concatenate([g("Dvy_T"), g("Cv_T")], 0)
    W0, W1, W2 = g("W0"), g("W1"), g("W2")
    Z = np.zeros_like
    blk = lambda A: np.block([[A, Z(A)], [Z(A), A]])
    return {
        "obsT16": obs.astype(np.float16),
        "x0T": x0T.astype(np.float32),
        "Wdvw": Dvw.astype(np.float16),
        "Wcd": Wcd.astype(np.float16),
        "Wcd0": (Wcd @ M).astype(np.float16),
        "Wu": np.concatenate([g("Duy_T"), g("Cu_T")], 0).astype(np.float16),
        "Wuw": g("Duw_T").astype(np.float16),
        "Wx": np.concatenate([DT * g("By_T"), DT * g("A_T")], 0).astype(np.float16),
        "Wxw": (DT * g("Bw_T")).astype(np.float16),
        "Wv0b": blk(W0).astype(np.float16),
        "Wv1b": blk(W1).astype(np.float16),
        "Wv2b": blk(W2).astype(np.float16),
        "b0v": np.tile(g("b0").reshape(H, 1), (2, 1)).astype(np.float32),
        "b1v": np.tile(g("b1").reshape(H, 1), (2, 1)).astype(np.float32),
    }


def assemble_output(results, inputs, n_cores=N_CORES):
    obs = np.asarray(inputs["obs"])
    Bfull, T = obs.shape[0], obs.shape[1]
    BL = Bfull // n_cores
    out = np.empty((Bfull, T, 2 * OUT + 1), np.float32)
    log_stds = np.asarray(inputs["log_stds"], np.float32)
    b2 = np.asarray(inputs["b2"], np.float32)
    for c in range(n_cores):
        sl = slice(c * BL, (c + 1) * BL)
        out[sl, :, :OUT] = results[c]["u_out"].transpose(2, 0, 1)
        out[sl, :, OUT:2 * OUT] = log_stds
        out[sl, :, 2 * OUT:] = results[c]["v_out"].T[:, :, None] + b2
    return out


_NC_CACHE = {}


def _get_nc(T):
    if T not in _NC_CACHE:
        _NC_CACHE[T] = build_kernel(T=T)
    return _NC_CACHE[T]


def run_on_hw(inputs, trace=False):
    """Run the SPMD kernel; returns (full_output, exec_time_ns_or_None)."""
    T = np.asarray(inputs["obs"]).shape[1]
    nc = _get_nc(T)
    in_maps = [host_inputs(inputs, c) for c in range(N_CORES)]
    last_err = None
    for attempt in range(3):
        try:
            res = run_bass_kernel_spmd(nc, in_maps, list(range(N_CORES)), trace=trace)
            return assemble_output(res.results, inputs), res.exec_time_ns
        except Exception as e:  # transient device failures: retry
            last_err = e
    raise last_err


def kernel(**inputs) -> np.ndarray:
    out, _ = run_on_hw(inputs, trace=False)
    return out


# revision 7
# speedup vs baseline: 3.4694x; 1.4082x over previous
"""DissipativeThetaRINN Trainium2 (Bass/Tile) kernel — 8-core data parallel.

Strategy (pure data parallel, per sharding hint):
  - Batch B=2048 is split across 8 NeuronCores (256 rows/core); the tiny
    controller matrices and value-MLP weights are replicated.
  - On-device layout is transposed: features on SBUF partitions, batch on
    the free dimension (256 columns per core).
  - The implicit layer w = tanh(Cv x + Dvy y + Dvw w) is solved with only
    N_ITERS=3 tanh evaluations: iteration 0 uses a linear-solve warm start
    w0 = tanh(M c) with M = (I - g Dvw)^-1, g=0.8, folded host-side into
    the const matmul (zero extra device work).  Each remaining iteration is
    one 256-wide matmul pair into PSUM + one 256-wide tanh.  All matmuls
    span the full 256-column batch — on TRN2 the per-instruction overhead
    (LDWEIGHTS ~100ns + drain ~170ns) dominates 128-col streams, so fewer,
    wider instructions beat a 2-chunk ping-pong.
  - x_next (forward Euler) uses the second-to-last w iterate so the final
    tanh is off the timestep-boundary critical path (error ~1e-5, verified
    in fp16-faithful numpy sim: total rel_l2 ≈ 5.5e-3 vs 2e-2 budget).
  - Matmuls run in fp16 (PSUM accumulates fp32); DT is pre-folded into the
    recurrence weights; the x accumulator stays fp32 on device.
  - The value MLP is computed in groups of 4 timesteps, packed 2-per-128
    partitions with block-diagonal weights, scheduled into engine gaps.
  - log_stds broadcast and the +b2 value bias are applied host-side.
"""
import numpy as np
import concourse.bass as bass
import concourse.mybir as mybir
import concourse.tile as tile
from concourse import bacc
from concourse.bass_utils import run_bass_kernel_spmd

dt = mybir.dt
AF = mybir.ActivationFunctionType

# problem shape (hardcoded per contract)
BFULL, TFULL = 2048, 128
S, NL, IN, OUT, H = 16, 128, 32, 8, 64
DT = 0.01
N_CORES = 8
N_ITERS = 3    # tanh evaluations per timestep (incl. warm-start iteration)
G_INIT = 0.8   # warm-start gain: w0 = tanh((I - g Dvw)^-T c)
VG = 4         # value-MLP timestep group (packed 2x2 onto 128 partitions)


def build_kernel(T=TFULL, B=BFULL // N_CORES, n_iters=N_ITERS):
    nc = bacc.Bacc(None, target_bir_lowering=False)
    f32, f16 = dt.float32, dt.float16
    assert n_iters >= 2

    obsT16 = nc.dram_tensor("obsT16", [T, IN, B], f16, kind="ExternalInput")
    x0T = nc.dram_tensor("x0T", [S, B], f32, kind="ExternalInput")
    Wdvw = nc.dram_tensor("Wdvw", [NL, NL], f16, kind="ExternalInput")
    Wcd = nc.dram_tensor("Wcd", [S + IN, NL], f16, kind="ExternalInput")
    Wcd0 = nc.dram_tensor("Wcd0", [S + IN, NL], f16, kind="ExternalInput")
    Wu = nc.dram_tensor("Wu", [S + IN, OUT], f16, kind="ExternalInput")
    Wuw = nc.dram_tensor("Wuw", [NL, OUT], f16, kind="ExternalInput")
    Wx = nc.dram_tensor("Wx", [S + IN, S], f16, kind="ExternalInput")
    Wxw = nc.dram_tensor("Wxw", [NL, S], f16, kind="ExternalInput")
    Wv0b = nc.dram_tensor("Wv0b", [2 * IN, 2 * H], f16, kind="ExternalInput")
    Wv1b = nc.dram_tensor("Wv1b", [2 * H, 2 * H], f16, kind="ExternalInput")
    Wv2b = nc.dram_tensor("Wv2b", [2 * H, 2], f16, kind="ExternalInput")
    b0v = nc.dram_tensor("b0v", [2 * H, 1], f32, kind="ExternalInput")
    b1v = nc.dram_tensor("b1v", [2 * H, 1], f32, kind="ExternalInput")

    u_out = nc.dram_tensor("u_out", [T, OUT, B], f32, kind="ExternalOutput")
    v_out = nc.dram_tensor("v_out", [T, B], f32, kind="ExternalOutput")

    B2 = 2 * B
    n_groups = (T + VG - 1) // VG

    with tile.TileContext(nc) as tc:
        with tc.tile_pool(name="wts", bufs=1) as wts, \
             tc.tile_pool(name="xyp", bufs=3) as xyp, \
             tc.tile_pool(name="xtp", bufs=2) as xtp, \
             tc.tile_pool(name="wp", bufs=3) as wp, \
             tc.tile_pool(name="obp", bufs=2) as obp, \
             tc.tile_pool(name="vp", bufs=2) as vp, \
             tc.tile_pool(name="up", bufs=2) as up, \
             tc.tile_pool(name="pw", bufs=2, space="PSUM") as pwp, \
             tc.tile_pool(name="px", bufs=2, space="PSUM") as pxp, \
             tc.tile_pool(name="pu", bufs=2, space="PSUM") as pup, \
             tc.tile_pool(name="ph", bufs=1, space="PSUM") as php, \
             tc.tile_pool(name="pv", bufs=1, space="PSUM") as pvp:

            def wt(name, dram, shape, dtp):
                tl = wts.tile(shape, dtp, name=name)
                nc.sync.dma_start(tl[:], dram[:])
                return tl
            wdvw = wt("wdvw", Wdvw, [NL, NL], f16)
            wcd = wt("wcd", Wcd, [S + IN, NL], f16)
            wcd0 = wt("wcd0", Wcd0, [S + IN, NL], f16)
            wu = wt("wu", Wu, [S + IN, OUT], f16)
            wuw = wt("wuw", Wuw, [NL, OUT], f16)
            wx = wt("wx", Wx, [S + IN, S], f16)
            wxw = wt("wxw", Wxw, [NL, S], f16)
            wv0b = wt("wv0b", Wv0b, [2 * IN, 2 * H], f16)
            wv1b = wt("wv1b", Wv1b, [2 * H, 2 * H], f16)
            wv2b = wt("wv2b", Wv2b, [2 * H, 2], f16)
            b0 = wt("b0", b0v, [2 * H, 1], f32)
            b1 = wt("b1", b1v, [2 * H, 1], f32)

            def load_obs4(g):
                """obs4 [64, 2B]: rows 0:32 = ts {4g, 4g+1}, rows 32:64 =
                ts {4g+2, 4g+3} (two col blocks of B each)."""
                t0 = g * VG
                ob = obp.tile([2 * IN, B2], f16, name=f"obs4_{g}", tag="obs4")
                nc.sync.dma_start(
                    ob[0:IN, :].rearrange("k (g1 b) -> k g1 b", g1=2),
                    obsT16[t0:t0 + 2].transpose([1, 0, 2]))
                nc.sync.dma_start(
                    ob[IN:, :].rearrange("k (g1 b) -> k g1 b", g1=2),
                    obsT16[t0 + 2:t0 + 4].transpose([1, 0, 2]))
                return ob

            obs4 = load_obs4(0)
            obs4_next = load_obs4(1) if n_groups > 1 else None

            # xy_h [48, B] f16: rows 0:32 = y^T, rows 32:48 = x^T
            xt_r = xtp.tile([S, B], f32, name="xt_r0", tag="xt_r")
            nc.sync.dma_start(xt_r[:], x0T[:])
            xy_h = xyp.tile([S + IN, B], f16, name="xy_h0", tag="xy_h")
            nc.sync.dma_start(xy_h[0:IN, :], obsT16[0])
            nc.vector.tensor_copy(xy_h[IN:, :], xt_r[:])

            u4 = None
            u_pend = None  # (pu_tile, w_final, t): wuw matmul deferred to t+1
            u_last = None

            def copy_u(tp):
                nonlocal u4
                g4p = tp % VG
                if g4p == 0:
                    u4 = up.tile([OUT, VG * B], f32, name=f"u4_{tp // VG}", tag="u4")
                nc.vector.tensor_copy(u4[:, g4p * B:(g4p + 1) * B], u_last[:])
                if g4p == VG - 1:
                    nc.sync.dma_start(
                        u_out[tp - VG + 1:tp + 1].transpose([1, 0, 2]),
                        u4[:].rearrange("o (g1 b) -> o g1 b", g1=VG))

            for t in range(T):
                g, g4 = t // VG, t % VG
                # -- PE front: warm-start + it1-const (need only xy), value
                #    layer-0, then t-1's deferred u close.
                p0 = pwp.tile([NL, B], f32, name=f"pw{t}_0", tag="pw")
                nc.tensor.matmul(p0[:], wcd0[:], xy_h[:], start=True, stop=True)
                p1 = pwp.tile([NL, B], f32, name=f"pw{t}_1", tag="pw")
                nc.tensor.matmul(p1[:], wcd[:], xy_h[:], start=True, stop=False)
                if g4 == 0:
                    ph = php.tile([2 * H, B2], f32, name=f"ph_{g}", tag="ph")
                    nc.tensor.matmul(ph[:], wv0b[:], obs4[:], start=True, stop=True)
                if u_pend is not None:
                    pu_p, w_p, _ = u_pend
                    nc.tensor.matmul(pu_p[:], wuw[:], w_p[:], start=False, stop=True)
                    u_last, u_pend = pu_p, None

                w0 = wp.tile([NL, B], f16, name=f"w{t}_0", tag="w")
                nc.scalar.activation(w0[:], p0[:], AF.Tanh)           # tanh0
                if g4 == 0:
                    h1 = vp.tile([2 * H, B2], f16, name=f"h1_{g}", tag="h")
                    nc.scalar.activation(h1[:], ph[:], AF.Tanh, bias=b0[:])

                # -- x/u const halves (need only xy), then the chain matmuls
                if t < T - 1:
                    px = pxp.tile([S, B], f32, name=f"px{t}", tag="px")
                    nc.tensor.matmul(px[:], wx[:], xy_h[:], start=True, stop=False)
                pu = pup.tile([OUT, B], f32, name=f"pu{t}", tag="pu")
                nc.tensor.matmul(pu[:], wu[:], xy_h[:], start=True, stop=False)

                nc.tensor.matmul(p1[:], wdvw[:], w0[:], start=False, stop=True)
                if t < T - 1:
                    # x_next from the warm start w0: frees the boundary chain
                    nc.tensor.matmul(px[:], wxw[:], w0[:], start=False, stop=True)
                    xy_hn = xyp.tile([S + IN, B], f16, name=f"xyh{t + 1}", tag="xy_h")
                    nc.sync.dma_start(xy_hn[0:IN, :], obsT16[t + 1])
                    nc.vector.tensor_add(xy_hn[IN:, :], px[:], xt_r[:])
                    if t < T - 2:
                        xt_rn = xtp.tile([S, B], f32, name=f"xtr{t + 1}", tag="xt_r")
                        nc.vector.tensor_add(xt_rn[:], px[:], xt_r[:])
                    else:
                        xt_rn = xt_r
                if u_last is not None and t > 0:
                    copy_u(t - 1)
                if g4 == 1:
                    ph2 = php.tile([2 * H, B2], f32, name=f"ph2_{g}", tag="ph")
                    nc.tensor.matmul(ph2[:], wv1b[:], h1[:], start=True, stop=True)

                # -- remaining fixed-point iterations (chain-paced)
                w_prev, pk = w0, p1
                for it in range(1, n_iters):
                    if it > 1:
                        pk = pwp.tile([NL, B], f32, name=f"pw{t}_{it}", tag="pw")
                        nc.tensor.matmul(pk[:], wcd[:], xy_h[:], start=True, stop=False)
                        nc.tensor.matmul(pk[:], wdvw[:], w_prev[:], start=False, stop=True)
                    wn = wp.tile([NL, B], f16, name=f"w{t}_{it}", tag="w")
                    nc.scalar.activation(wn[:], pk[:], AF.Tanh)
                    w_prev = wn
                    if it == 1 and g4 == 1:
                        h2 = vp.tile([2 * H, B2], f16, name=f"h2_{g}", tag="h")
                        nc.scalar.activation(h2[:], ph2[:], AF.Tanh, bias=b1[:])

                # defer u's wuw matmul (needs final w) into t+1's PE queue
                u_pend = (pu, w_prev, t)

                if g4 == 2:
                    pv = pvp.tile([2, B2], f32, name=f"pv_{g}", tag="pv")
                    nc.tensor.matmul(pv[:], wv2b[:], h2[:], start=True, stop=True)
                    v_sb = vp.tile([2, B2], f32, name=f"v_sb{g}", tag="v_sb")
                    nc.vector.tensor_copy(v_sb[:], pv[:])
                    nc.sync.dma_start(
                        v_out[t - 2:t + 2].rearrange("(r g1) b -> r (g1 b)", r=2),
                        v_sb[:])
                if g4 == VG - 1:
                    # rotate prefetched obs group
                    obs4 = obs4_next
                    if g + 2 < n_groups:
                        obs4_next = load_obs4(g + 2)

                if t < T - 1:
                    xt_r, xy_h = xt_rn, xy_hn

            # close out the final timestep's u
            pu_p, w_p, _ = u_pend
            nc.tensor.matmul(pu_p[:], wuw[:], w_p[:], start=False, stop=True)
            u_last = pu_p
            copy_u(T - 1)

    nc.compile()
    return nc


def host_inputs(inputs, core, n_cores=N_CORES):
    BL = inputs["obs"].shape[0] // n_cores
    sl = slice(core * BL, (core + 1) * BL)
    obs = np.ascontiguousarray(np.asarray(inputs["obs"])[sl].transpose(1, 2, 0))
    x0T = np.ascontiguousarray(np.asarray(inputs["x0"])[sl].T)
    g = lambda k: np.asarray(inputs[k]).astype(np.float32)
    Dvw = g("Dvw_T")
    M = np.linalg.inv(np.eye(NL, dtype=np.float32) - G_INIT * Dvw)
    Wcd = np.concatenate([g("Dvy_T"), g("Cv_T")], 0)
    W0, W1, W2 = g("W0"), g("W1"), g("W2")
    Z = np.zeros_like
    blk = lambda A: np.block([[A, Z(A)], [Z(A), A]])
    return {
        "obsT16": obs.astype(np.float16),
        "x0T": x0T.astype(np.float32),
        "Wdvw": Dvw.astype(np.float16),
        "Wcd": Wcd.astype(np.float16),
        "Wcd0": (Wcd @ M).astype(np.float16),
        "Wu": np.concatenate([g("Duy_T"), g("Cu_T")], 0).astype(np.float16),
        "Wuw": g("Duw_T").astype(np.float16),
        "Wx": np.concatenate([DT * g("By_T"), DT * g("A_T")], 0).astype(np.float16),
        "Wxw": (DT * g("Bw_T")).astype(np.float16),
        "Wv0b": blk(W0).astype(np.float16),
        "Wv1b": blk(W1).astype(np.float16),
        "Wv2b": blk(W2).astype(np.float16),
        "b0v": np.tile(g("b0").reshape(H, 1), (2, 1)).astype(np.float32),
        "b1v": np.tile(g("b1").reshape(H, 1), (2, 1)).astype(np.float32),
    }


def assemble_output(results, inputs, n_cores=N_CORES):
    obs = np.asarray(inputs["obs"])
    Bfull, T = obs.shape[0], obs.shape[1]
    BL = Bfull // n_cores
    out = np.empty((Bfull, T, 2 * OUT + 1), np.float32)
    log_stds = np.asarray(inputs["log_stds"], np.float32)
    b2 = np.asarray(inputs["b2"], np.float32)
    for c in range(n_cores):
        sl = slice(c * BL, (c + 1) * BL)
        out[sl, :, :OUT] = results[c]["u_out"].transpose(2, 0, 1)
        out[sl, :, OUT:2 * OUT] = log_stds
        out[sl, :, 2 * OUT:] = results[c]["v_out"].T[:, :, None] + b2
    return out


_NC_CACHE = {}


def _get_nc(T):
    if T not in _NC_CACHE:
        _NC_CACHE[T] = build_kernel(T=T)
    return _NC_CACHE[T]


def run_on_hw(inputs, trace=False):
    """Run the SPMD kernel; returns (full_output, exec_time_ns_or_None)."""
    T = np.asarray(inputs["obs"]).shape[1]
    nc = _get_nc(T)
    in_maps = [host_inputs(inputs, c) for c in range(N_CORES)]
    last_err = None
    for attempt in range(3):
        try:
            res = run_bass_kernel_spmd(nc, in_maps, list(range(N_CORES)), trace=trace)
            return assemble_output(res.results, inputs), res.exec_time_ns
        except Exception as e:  # transient device failures: retry
            last_err = e
    raise last_err


def kernel(**inputs) -> np.ndarray:
    out, _ = run_on_hw(inputs, trace=False)
    return out


# revision 10
# speedup vs baseline: 3.5823x; 1.0325x over previous
"""DissipativeThetaRINN Trainium2 (Bass/Tile) kernel — 8-core data parallel.

Strategy (pure data parallel, per sharding hint):
  - Batch B=2048 is split across 8 NeuronCores (256 rows/core); the tiny
    controller matrices and value-MLP weights are replicated.
  - On-device layout is transposed: features on SBUF partitions, batch on
    the free dimension (256 columns per core).
  - The implicit layer w = tanh(Cv x + Dvy y + Dvw w) is solved with only
    N_ITERS=3 tanh evaluations: iteration 0 uses a linear-solve warm start
    w0 = tanh(M c) with M = (I - g Dvw)^-1, g=0.8, folded host-side into
    the const matmul (zero extra device work).  Each remaining iteration is
    one 256-wide matmul pair into PSUM + one 256-wide tanh.  All matmuls
    span the full 256-column batch — on TRN2 the per-instruction overhead
    (LDWEIGHTS ~100ns + drain ~170ns) dominates 128-col streams, so fewer,
    wider instructions beat a 2-chunk ping-pong.
  - x_next (forward Euler) uses the second-to-last w iterate so the final
    tanh is off the timestep-boundary critical path (error ~1e-5, verified
    in fp16-faithful numpy sim: total rel_l2 ≈ 5.5e-3 vs 2e-2 budget).
  - Matmuls run in fp16 (PSUM accumulates fp32); DT is pre-folded into the
    recurrence weights; the x accumulator stays fp32 on device.
  - The value MLP is computed in groups of 4 timesteps, packed 2-per-128
    partitions with block-diagonal weights, scheduled into engine gaps.
  - log_stds broadcast and the +b2 value bias are applied host-side.
"""
import numpy as np
import concourse.bass as bass
import concourse.mybir as mybir
import concourse.tile as tile
from concourse import bacc
from concourse.bass_utils import run_bass_kernel_spmd

dt = mybir.dt
AF = mybir.ActivationFunctionType

# problem shape (hardcoded per contract)
BFULL, TFULL = 2048, 128
S, NL, IN, OUT, H = 16, 128, 32, 8, 64
DT = 0.01
N_CORES = 8
N_ITERS = 3    # tanh evaluations per timestep (incl. warm-start iteration)
G_INIT = 0.8   # warm-start gain: w0 = tanh((I - g Dvw)^-T c)
VG = 4         # value-MLP timestep group (packed 2x2 onto 128 partitions)


def build_kernel(T=TFULL, B=BFULL // N_CORES, n_iters=N_ITERS):
    nc = bacc.Bacc(None, target_bir_lowering=False)
    f32, f16 = dt.float32, dt.float16
    assert n_iters >= 2

    obsT16 = nc.dram_tensor("obsT16", [T, IN, B], f16, kind="ExternalInput")
    x0T = nc.dram_tensor("x0T", [S, B], f32, kind="ExternalInput")
    Wdvw = nc.dram_tensor("Wdvw", [NL, NL], f16, kind="ExternalInput")
    Wcd = nc.dram_tensor("Wcd", [S + IN, NL], f16, kind="ExternalInput")
    Wcd0 = nc.dram_tensor("Wcd0", [S + IN, NL], f16, kind="ExternalInput")
    Wxu = nc.dram_tensor("Wxu", [S + IN, 32 + OUT], f16, kind="ExternalInput")
    Wuw = nc.dram_tensor("Wuw", [NL, OUT], f16, kind="ExternalInput")
    Wxw = nc.dram_tensor("Wxw", [NL, S], f16, kind="ExternalInput")
    Wv0b = nc.dram_tensor("Wv0b", [2 * IN, 2 * H], f16, kind="ExternalInput")
    Wv1b = nc.dram_tensor("Wv1b", [2 * H, 2 * H], f16, kind="ExternalInput")
    Wv2b = nc.dram_tensor("Wv2b", [2 * H, 2], f16, kind="ExternalInput")
    b0v = nc.dram_tensor("b0v", [2 * H, 1], f32, kind="ExternalInput")
    b1v = nc.dram_tensor("b1v", [2 * H, 1], f32, kind="ExternalInput")

    u_out = nc.dram_tensor("u_out", [T, OUT, B], f32, kind="ExternalOutput")
    v_out = nc.dram_tensor("v_out", [T, B], f32, kind="ExternalOutput")

    B2 = 2 * B
    n_groups = (T + VG - 1) // VG

    with tile.TileContext(nc) as tc:
        with tc.tile_pool(name="wts", bufs=1) as wts, \
             tc.tile_pool(name="xyp", bufs=3) as xyp, \
             tc.tile_pool(name="xtp", bufs=2) as xtp, \
             tc.tile_pool(name="wp", bufs=3) as wp, \
             tc.tile_pool(name="obp", bufs=2) as obp, \
             tc.tile_pool(name="vp", bufs=2) as vp, \
             tc.tile_pool(name="up", bufs=2) as up, \
             tc.tile_pool(name="pw", bufs=2, space="PSUM") as pwp, \
             tc.tile_pool(name="pxu", bufs=2, space="PSUM") as pxup, \
             tc.tile_pool(name="ph", bufs=1, space="PSUM") as php, \
             tc.tile_pool(name="pv", bufs=1, space="PSUM") as pvp:

            def wt(name, dram, shape, dtp):
                tl = wts.tile(shape, dtp, name=name)
                nc.sync.dma_start(tl[:], dram[:])
                return tl
            wdvw = wt("wdvw", Wdvw, [NL, NL], f16)
            wcd = wt("wcd", Wcd, [S + IN, NL], f16)
            wcd0 = wt("wcd0", Wcd0, [S + IN, NL], f16)
            wxu = wt("wxu", Wxu, [S + IN, 32 + OUT], f16)
            wuw = wt("wuw", Wuw, [NL, OUT], f16)
            wxw = wt("wxw", Wxw, [NL, S], f16)
            wv0b = wt("wv0b", Wv0b, [2 * IN, 2 * H], f16)
            wv1b = wt("wv1b", Wv1b, [2 * H, 2 * H], f16)
            wv2b = wt("wv2b", Wv2b, [2 * H, 2], f16)
            b0 = wt("b0", b0v, [2 * H, 1], f32)
            b1 = wt("b1", b1v, [2 * H, 1], f32)

            def load_obs4(g):
                """obs4 [64, 2B]: rows 0:32 = ts {4g, 4g+1}, rows 32:64 =
                ts {4g+2, 4g+3} (two col blocks of B each)."""
                t0 = g * VG
                ob = obp.tile([2 * IN, B2], f16, name=f"obs4_{g}", tag="obs4")
                nc.sync.dma_start(
                    ob[0:IN, :].rearrange("k (g1 b) -> k g1 b", g1=2),
                    obsT16[t0:t0 + 2].transpose([1, 0, 2]))
                nc.sync.dma_start(
                    ob[IN:, :].rearrange("k (g1 b) -> k g1 b", g1=2),
                    obsT16[t0 + 2:t0 + 4].transpose([1, 0, 2]))
                return ob

            obs4 = load_obs4(0)
            obs4_next = load_obs4(1) if n_groups > 1 else None

            # xy_h [48, B] f16: rows 0:32 = y^T, rows 32:48 = x^T
            xt_r = xtp.tile([S, B], f32, name="xt_r0", tag="xt_r")
            nc.sync.dma_start(xt_r[:], x0T[:])
            xy_h = xyp.tile([S + IN, B], f16, name="xy_h0", tag="xy_h")
            nc.sync.dma_start(xy_h[0:IN, :], obsT16[0])
            nc.vector.tensor_copy(xy_h[IN:, :], xt_r[:])

            u4 = None
            u_pend = None  # (pu_tile, w_final, t): wuw matmul deferred to t+1
            u_last = None

            def copy_u(tp):
                nonlocal u4
                g4p = tp % VG
                if g4p == 0:
                    u4 = up.tile([OUT, VG * B], f32, name=f"u4_{tp // VG}", tag="u4")
                nc.vector.tensor_copy(u4[:, g4p * B:(g4p + 1) * B], u_last)
                if g4p == VG - 1:
                    nc.sync.dma_start(
                        u_out[tp - VG + 1:tp + 1].transpose([1, 0, 2]),
                        u4[:].rearrange("o (g1 b) -> o g1 b", g1=VG))

            for t in range(T):
                g, g4 = t // VG, t % VG
                # -- PE front: warm-start + it1-const (need only xy), value
                #    layer-0, then t-1's deferred u close.
                p0 = pwp.tile([NL, B], f32, name=f"pw{t}_0", tag="pw")
                nc.tensor.matmul(p0[:], wcd0[:], xy_h[:], start=True, stop=True)
                p1 = pwp.tile([NL, B], f32, name=f"pw{t}_1", tag="pw")
                nc.tensor.matmul(p1[:], wcd[:], xy_h[:], start=True, stop=False)
                if g4 == 0:
                    ph = php.tile([2 * H, B2], f32, name=f"ph_{g}", tag="ph")
                    nc.tensor.matmul(ph[:], wv0b[:], obs4[:], start=True, stop=True)
                if u_pend is not None:
                    pu_p, w_p, _ = u_pend
                    nc.tensor.matmul(pu_p, wuw[:], w_p[:], start=False, stop=True,
                                     skip_group_check=True)
                    u_last, u_pend = pu_p, None

                w0 = wp.tile([NL, B], f16, name=f"w{t}_0", tag="w")
                nc.scalar.activation(w0[:], p0[:], AF.Tanh)           # tanh0
                if g4 == 0:
                    h1 = vp.tile([2 * H, B2], f16, name=f"h1_{g}", tag="h")
                    nc.scalar.activation(h1[:], ph[:], AF.Tanh, bias=b0[:])

                # -- x/u const halves fused into one matmul (rows 0:S = x,
                #    rows S:S+OUT = u), then the chain matmuls
                pxu = pxup.tile([32 + OUT, B], f32, name=f"pxu{t}", tag="pxu")
                nc.tensor.matmul(pxu[:], wxu[:], xy_h[:], start=True, stop=False)
                px = pxu[0:S, :]
                pu = pxu[32:, :]

                nc.tensor.matmul(p1[:], wdvw[:], w0[:], start=False, stop=True)
                if t < T - 1:
                    # x_next from the warm start w0: frees the boundary chain
                    nc.tensor.matmul(px, wxw[:], w0[:], start=False, stop=True,
                                     skip_group_check=True)
                    xy_hn = xyp.tile([S + IN, B], f16, name=f"xyh{t + 1}", tag="xy_h")
                    nc.sync.dma_start(xy_hn[0:IN, :], obsT16[t + 1])
                    nc.vector.tensor_add(xy_hn[IN:, :], px, xt_r[:])
                    if t < T - 2:
                        xt_rn = xtp.tile([S, B], f32, name=f"xtr{t + 1}", tag="xt_r")
                        nc.vector.tensor_add(xt_rn[:], px, xt_r[:])
                    else:
                        xt_rn = xt_r
                if u_last is not None and t > 0:
                    copy_u(t - 1)
                if g4 == 1:
                    ph2 = php.tile([2 * H, B2], f32, name=f"ph2_{g}", tag="ph")
                    nc.tensor.matmul(ph2[:], wv1b[:], h1[:], start=True, stop=True)

                # -- remaining fixed-point iterations (chain-paced)
                w_prev, pk = w0, p1
                for it in range(1, n_iters):
                    if it > 1:
                        pk = pwp.tile([NL, B], f32, name=f"pw{t}_{it}", tag="pw")
                        nc.tensor.matmul(pk[:], wcd[:], xy_h[:], start=True, stop=False)
                        nc.tensor.matmul(pk[:], wdvw[:], w_prev[:], start=False, stop=True)
                    wn = wp.tile([NL, B], f16, name=f"w{t}_{it}", tag="w")
                    nc.scalar.activation(wn[:], pk[:], AF.Tanh)
                    w_prev = wn
                    if it == 1 and g4 == 1:
                        h2 = vp.tile([2 * H, B2], f16, name=f"h2_{g}", tag="h")
                        nc.scalar.activation(h2[:], ph2[:], AF.Tanh, bias=b1[:])

                # defer u's wuw matmul (needs final w) into t+1's PE queue
                u_pend = (pu, w_prev, t)

                if g4 == 2:
                    pv = pvp.tile([2, B2], f32, name=f"pv_{g}", tag="pv")
                    nc.tensor.matmul(pv[:], wv2b[:], h2[:], start=True, stop=True)
                    v_sb = vp.tile([2, B2], f32, name=f"v_sb{g}", tag="v_sb")
                    nc.vector.tensor_copy(v_sb[:], pv[:])
                    nc.sync.dma_start(
                        v_out[t - 2:t + 2].rearrange("(r g1) b -> r (g1 b)", r=2),
                        v_sb[:])
                if g4 == VG - 1:
                    # rotate prefetched obs group
                    obs4 = obs4_next
                    if g + 2 < n_groups:
                        obs4_next = load_obs4(g + 2)

                if t < T - 1:
                    xt_r, xy_h = xt_rn, xy_hn

            # close out the final timestep's u
            pu_p, w_p, _ = u_pend
            nc.tensor.matmul(pu_p, wuw[:], w_p[:], start=False, stop=True,
                             skip_group_check=True)
            u_last = pu_p
            copy_u(T - 1)

    nc.compile()
    return nc


def host_inputs(inputs, core, n_cores=N_CORES):
    BL = inputs["obs"].shape[0] // n_cores
    sl = slice(core * BL, (core + 1) * BL)
    obs = np.ascontiguousarray(np.asarray(inputs["obs"])[sl].transpose(1, 2, 0))
    x0T = np.ascontiguousarray(np.asarray(inputs["x0"])[sl].T)
    g = lambda k: np.asarray(inputs[k]).astype(np.float32)
    Dvw = g("Dvw_T")
    M = np.linalg.inv(np.eye(NL, dtype=np.float32) - G_INIT * Dvw)
    Wcd = np.concatenate([g("Dvy_T"), g("Cv_T")], 0)
    W0, W1, W2 = g("W0"), g("W1"), g("W2")
    Z = np.zeros_like
    blk = lambda A: np.block([[A, Z(A)], [Z(A), A]])
    return {
        "obsT16": obs.astype(np.float16),
        "x0T": x0T.astype(np.float32),
        "Wdvw": Dvw.astype(np.float16),
        "Wcd": Wcd.astype(np.float16),
        "Wcd0": (Wcd @ M).astype(np.float16),
        "Wxu": np.concatenate(
            [np.concatenate([DT * g("By_T"), DT * g("A_T")], 0),
             np.zeros((S + IN, 32 - S), np.float32),
             np.concatenate([g("Duy_T"), g("Cu_T")], 0)], 1).astype(np.float16),
        "Wuw": g("Duw_T").astype(np.float16),
        "Wxw": (DT * g("Bw_T")).astype(np.float16),
        "Wv0b": blk(W0).astype(np.float16),
        "Wv1b": blk(W1).astype(np.float16),
        "Wv2b": blk(W2).astype(np.float16),
        "b0v": np.tile(g("b0").reshape(H, 1), (2, 1)).astype(np.float32),
        "b1v": np.tile(g("b1").reshape(H, 1), (2, 1)).astype(np.float32),
    }


def assemble_output(results, inputs, n_cores=N_CORES):
    obs = np.asarray(inputs["obs"])
    Bfull, T = obs.shape[0], obs.shape[1]
    BL = Bfull // n_cores
    out = np.empty((Bfull, T, 2 * OUT + 1), np.float32)
    log_stds = np.asarray(inputs["log_stds"], np.float32)
    b2 = np.asarray(inputs["b2"], np.float32)
    for c in range(n_cores):
        sl = slice(c * BL, (c + 1) * BL)
        out[sl, :, :OUT] = results[c]["u_out"].transpose(2, 0, 1)
        out[sl, :, OUT:2 * OUT] = log_stds
        out[sl, :, 2 * OUT:] = results[c]["v_out"].T[:, :, None] + b2
    return out


_NC_CACHE = {}


def _get_nc(T):
    if T not in _NC_CACHE:
        _NC_CACHE[T] = build_kernel(T=T)
    return _NC_CACHE[T]


def run_on_hw(inputs, trace=False):
    """Run the SPMD kernel; returns (full_output, exec_time_ns_or_None)."""
    T = np.asarray(inputs["obs"]).shape[1]
    nc = _get_nc(T)
    in_maps = [host_inputs(inputs, c) for c in range(N_CORES)]
    last_err = None
    for attempt in range(3):
        try:
            res = run_bass_kernel_spmd(nc, in_maps, list(range(N_CORES)), trace=trace)
            return assemble_output(res.results, inputs), res.exec_time_ns
        except Exception as e:  # transient device failures: retry
            last_err = e
    raise last_err


def kernel(**inputs) -> np.ndarray:
    out, _ = run_on_hw(inputs, trace=False)
    return out
